# revision 12
# baseline (speedup 1.0000x reference)
"""Trainium2 Bass kernel: GNN conv block (nn_Conv_block_49331994362308).

Computes, for N=100000 nodes with K=16 neighbors each:
    nh  = ij[:, :, 0]                      # [N, K] neighbor ids
    xnj = mean(x[nh], axis=1)              # neighbor-feature mean  [N, 128]
    xej = mean(e, axis=1)                  # edge-feature mean      [N, 64]
    out = relu(x @ Wc.T + xnj @ Wn.T + xej @ We.T)

Distribution: data-parallel over nodes across 8 NeuronCores (12500 nodes
per core, padded to 12544 = 98*128). x is replicated to every core (bf16)
so the random neighbor gather x[nh] is a core-local indirect DMA from HBM.

v2 (vs the f32 baseline):
  - Everything device-side is bf16 (host pre-casts; 1/K folded into the
    weights; output bf16, host casts back to f32). Halves e/x/out DMA.
  - The gather pulls bf16 rows (256B elems) directly - no ACT cast.
  - Gather groups of GRP=7 tiles: 4 SWDGE instructions per 7 tiles
    (fixed ~1us Q7 cost per instruction dominates, so fewer is better).
  - No PE transposes: x ships pre-transposed (xT_loc), e ships in a
    (parity*feature, kk, node) layout whose DVE reduce directly yields
    the final-matmul lhsT, with weT2 = [We.T; We.T]/K contracting both
    parity halves; xnj pooling already yields [feature, node].
  - One-hot pooling matrices are built ON DEVICE: host ships per-slot
    node ids (int16, 255=pad), one DVE is_equal per tile expands them
    against a constant iota to the fp8 [slot, node] one-hot.

Per-core pipeline per 128-node tile:
  Pool: (per 7-tile group) 4x dma_gather of class nh%4 super-rows.
  DVE:  is_equal one-hot build; e-mean via strided tensor_reduce.
  PE:   20 accumulating bf16xfp8 pool matmuls (xnjT = sum x[nh].T),
        then 3 accumulating bf16 matmuls against the weights.
  ACT:  xnjT PSUM->bf16 cast; ReLU into the per-14-tile staging buffer.

Walrus's TRN2 queue-DMA codegen only supports ONE sync-wait command per
DMA (and one per PE LDWEIGHTS), so the structure keeps every DMA at a
single dependency front: indices/nodeids are preloaded once into SBUF,
the 8 SWDGE bookkeeping lanes are warmed with dummy transfers that
absorb the preload front, and outputs go to once-written per-chunk DRAM
tensors (no WAW chains). _legalize_waits moves any residual extra waits
onto no-op carrier instructions.
"""

from contextlib import ExitStack

import numpy as np

import concourse.bass as bass
import concourse.mybir as mybir
import concourse.tile as tile
from concourse.bass_utils import run_bass_kernel_spmd
from concourse import library_config

P = 128
K = 16
XN_IN = 128
XE_IN = 64
XN_OUT = 128
N_CORES = 8
N_FULL = 100000
N_LOC = N_FULL // N_CORES          # 12500
N_LOC_PAD = ((N_LOC + P - 1) // P) * P  # 12544
CHUNK = 14                          # tiles per output chunk (98 = 7*14)

F32 = mybir.dt.float32
BF16 = mybir.dt.bfloat16
F8 = mybir.dt.float8e4   # one-hot pooling matrices hold only 0/1 - exact
I16 = mybir.dt.int16

GRP = 7            # tiles per gather group (must divide CHUNK)
NCLS = 4           # x rows per int16 "super-row" (mod classes)
SEG = 640          # padded gather slots per (tile, class); 5 chunks of 128
NCH = SEG // P     # chunks per (tile, class) = 5
CH_T = NCH * NCLS  # pool chunks per tile = 20


def _chunks(n_tiles: int) -> list[int]:
    out = []
    t = 0
    while t < n_tiles:
        out.append(min(CHUNK, n_tiles - t))
        t += CHUNK
    return out


def build_program(n_loc_pad: int, n_src: int) -> bass.Bass:
    """Build the SPMD per-core Bass program (same program on every core)."""
    assert n_loc_pad % P == 0
    n_tiles = n_loc_pad // P
    chunks = _chunks(n_tiles)

    # detect_race_conditions=False: the post-schedule wait-legalizer's nop
    # carriers share scratch tiles and trip the sim race detector's
    # bookkeeping (same-engine program order makes them safe).
    nc = bass.Bass("TRN2", debug=False, detect_race_conditions=False)

    grp = GRP if n_tiles % GRP == 0 else (2 if n_tiles % 2 == 0 else 1)
    n_groups = n_tiles // grp
    seg_i16 = grp * SEG // 16  # idx16 columns per (group, class)

    x4 = nc.dram_tensor("x4", [n_src // NCLS, NCLS * XN_IN], BF16,
                        kind="ExternalInput").ap()
    xT_loc = nc.dram_tensor("xT_loc", [P, n_tiles * P], BF16,
                            kind="ExternalInput").ap()
    eT_loc = nc.dram_tensor("eT_loc", [P, n_tiles * (K // 2) * P], BF16,
                            kind="ExternalInput").ap()
    # int16 super-row ids (nh//4), wrapped [16, L/16] + replicated to 128
    # partitions, concatenated over (group, class)
    idx_loc = nc.dram_tensor(
        "idx_loc", [P, n_groups * NCLS * seg_i16], I16, kind="ExternalInput"
    ).ap()
    # per-slot node ids (0..127, 255=pad): [128 slot, tile*CH_T chunks]
    nid_loc = nc.dram_tensor(
        "nid_loc", [P, n_tiles * CH_T], I16, kind="ExternalInput"
    ).ap()
    wcT = nc.dram_tensor("wcT", [XN_IN, XN_OUT], BF16, kind="ExternalInput").ap()
    wnT = nc.dram_tensor("wnT", [XN_IN, XN_OUT], BF16, kind="ExternalInput").ap()
    weT2 = nc.dram_tensor("weT2", [P, XN_OUT], BF16, kind="ExternalInput").ap()
    # per-chunk outputs, partition-major: out_c[p, i*128+f] = out[(t0+i)*128+p, f]
    outs = [
        nc.dram_tensor(f"out{c}", [P, ct * XN_OUT], BF16, kind="ExternalOutput").ap()
        for c, ct in enumerate(chunks)
    ]

    nop_sem = nc.alloc_semaphore("waitnop")

    with tile.TileContext(nc) as tc, ExitStack() as ctx:
        nc.gpsimd.sem_clear(range(nop_sem.num, nop_sem.num + 1))
        consts = ctx.enter_context(tc.tile_pool(name="consts", bufs=1))
        # iota_t[p, c*128+n] = n  (for the is_equal one-hot expansion).
        # Emitted before load_library: Iota lives in the 'standard' Pool
        # library, dma_gather in 'mlp'.
        iota_t = consts.tile([P, CH_T * P], I16, tag="iota_t")
        nc.gpsimd.iota(
            iota_t[:].rearrange("p (c n) -> p c n", n=P),
            pattern=[[0, CH_T], [1, P]],
            channel_multiplier=0,
        )
        nc.gpsimd.load_library(library_config.mlp)
        wcT_sb = consts.tile([XN_IN, XN_OUT], BF16, tag="wc")
        wnT_sb = consts.tile([XN_IN, XN_OUT], BF16, tag="wn")
        weT2_sb = consts.tile([P, XN_OUT], BF16, tag="we2")
        nc.sync.dma_start(wcT_sb[:], wcT[:, :])
        nc.sync.dma_start(wnT_sb[:], wnT[:, :])
        nc.sync.dma_start(weT2_sb[:], weT2[:, :])
        idx_all = consts.tile([P, n_groups * NCLS * seg_i16], I16, tag="idx_all")
        nc.sync.dma_start(idx_all[:], idx_loc[:, :])
        nid_all = consts.tile([P, n_tiles * CH_T], I16, tag="nid_all")
        nc.sync.dma_start(nid_all[:], nid_loc[:, :])
        # x viewed as [n_src/4, 4, 128] bf16: class j gathers row 4*i16+j via
        # elem_step=512 elements (1024B stride) and a j*128-element offset
        x4v = x4.rearrange("r (c f) -> r c f", c=NCLS)

        # Warm the 8 SWDGE bookkeeping lanes: each dummy absorbs the
        # idx-preload front so later gathers carry only their PE front.
        scratch = ctx.enter_context(tc.tile_pool(name="scratch", bufs=1))
        for q in range(8):
            sc = scratch.tile([1, K], I16, tag=f"sc{q}")
            nc.gpsimd.dma_start(sc[:], idx_all[:1, :K])
        # Tiny template instructions for _legalize_waits nop carriers
        # (one per DMA queue and per compute engine).
        nop_hw = scratch.tile([1, K], I16, tag="noptpl_hw")
        nc.sync.dma_start(nop_hw[:], idx_loc[:1, :K])
        nop_sw = scratch.tile([1, K], I16, tag="noptpl_sw")
        nc.gpsimd.dma_start(nop_sw[:], idx_loc[:1, :K])
        nop_dve = scratch.tile([P, K], I16, tag="noptpl_dve")
        nc.vector.tensor_copy(nop_dve[:], iota_t[:, :K])
        nop_act = scratch.tile([P, K], I16, tag="noptpl_act")
        nc.scalar.copy(nop_act[:], iota_t[:, :K])
        nop_pool = scratch.tile([P, K], F32, tag="noptpl_pool")
        nc.gpsimd.memset(nop_pool[:], 0.0)

        g_pool = ctx.enter_context(tc.tile_pool(name="gatherp", bufs=2))
        pp_pool = ctx.enter_context(tc.tile_pool(name="poolmat", bufs=3))
        e_pool = ctx.enter_context(tc.tile_pool(name="edgep", bufs=2))
        xs_pool = ctx.enter_context(tc.tile_pool(name="xselfp", bufs=2))
        st_pool = ctx.enter_context(tc.tile_pool(name="stagep", bufs=3))
        out_pool = ctx.enter_context(tc.tile_pool(name="outp", bufs=2))
        psum_pool = ctx.enter_context(tc.tile_pool(name="psump", bufs=2, space="PSUM"))
        psum1_pool = ctx.enter_context(tc.tile_pool(name="psum1p", bufs=2, space="PSUM"))

        # Warm up PE's view of the constant weights so steady-state matmuls
        # carry at most one sync wait (PE LDWEIGHTS supports a single wait).
        ps_warm = psum1_pool.tile([P, P], F32, tag="warm")
        nc.tensor.matmul(ps_warm[:], wcT_sb[:], wcT_sb[:], start=True, stop=False)
        nc.tensor.matmul(ps_warm[:], wnT_sb[:], wnT_sb[:], start=False, stop=False)
        nc.tensor.matmul(ps_warm[:], weT2_sb[:], weT2_sb[:], start=False, stop=True)

        t = 0
        gbf = [None] * NCLS
        nidx_reg = nc.gpsimd.to_reg(grp * SEG)  # shared across all gathers
        for c, ct in enumerate(chunks):
            o_stage = out_pool.tile([P, ct * XN_OUT], BF16, tag="ostage")
            for i in range(ct):
                g, ti = divmod(t, grp)

                if ti == 0:
                    # per-group gathers: one dma_gather per mod-4 class of
                    # grp*SEG slots; slot s lands at partition s%128, free
                    # block s//128, so 128-slot chunks stay within one tile.
                    for j in range(NCLS):
                        off = (g * NCLS + j) * seg_i16
                        gb = g_pool.tile(
                            [P, grp * SEG // P, XN_IN], BF16, tag=f"gb{j}"
                        )
                        nc.gpsimd.dma_gather(
                            out_ap=gb[:],
                            in_ap=x4v[:, j, :],
                            idxs_ap=idx_all[:, off:off + seg_i16],
                            num_idxs=grp * SEG,
                            num_idxs_reg=nidx_reg,
                            elem_size=XN_IN,
                            elem_step=NCLS * XN_IN,
                            single_packet=False,
                        )
                        gbf[j] = gb[:].rearrange("p b f -> p (b f)")

                # one-hot P[slot, n] = (nid[slot, chunk] == n), bf16
                p_sb = pp_pool.tile([P, CH_T * P], BF16, tag="pmat")
                nid_b = nid_all[:, t * CH_T:(t + 1) * CH_T]
                in0, in1 = bass.broadcast_tensor_aps(
                    iota_t[:].rearrange("p (c n) -> p c n", n=P),
                    nid_b.rearrange("p (c one) -> p c one", one=1),
                )
                nc.vector.tensor_tensor(
                    p_sb[:].rearrange("p (c n) -> p c n", n=P),
                    in0, in1, op=mybir.AluOpType.is_equal,
                )

                # e arrives pre-permuted: e_sb[p=(par,f), kk*128+n]; the mean
                # over k happens on PE as 8 extra accumulating matmuls below.
                # One DMA per group (HWDGE issue cost dominates small DMAs).
                if ti == 0:
                    e_sb = e_pool.tile([P, grp * (K // 2) * P], BF16, tag="e")
                    nc.sync.dma_start(
                        e_sb[:],
                        eT_loc[:, t * (K // 2) * P:(t + grp) * (K // 2) * P],
                    )
                    xT_sb = xs_pool.tile([P, grp * P], BF16, tag="xT")
                    nc.sync.dma_start(
                        xT_sb[:], xT_loc[:, t * P:(t + grp) * P]
                    )
                e_off = ti * (K // 2) * P

                # xnjT[f, n] = sum_slot g[slot, f] * P[slot, n]
                xnjT_ps = psum_pool.tile([P, P], F32, tag="ps_xnj")
                for b in range(CH_T):
                    j, bl = divmod(b, NCH)
                    blk = ti * NCH + bl
                    nc.tensor.matmul(
                        xnjT_ps[:],
                        gbf[j][:, blk * XN_IN:(blk + 1) * XN_IN],
                        p_sb[:, b * P:(b + 1) * P],
                        start=(b == 0),
                        stop=(b == CH_T - 1),
                    )
                xnjT_sb = st_pool.tile([P, P], BF16, tag="sb_xnj")
                nc.scalar.copy(xnjT_sb[:], xnjT_ps[:])

                out_ps = psum1_pool.tile([P, XN_OUT], F32, tag="ps_out")
                nc.tensor.matmul(
                    out_ps[:], xT_sb[:, ti * P:(ti + 1) * P], wcT_sb[:],
                    start=True, stop=False,
                )
                nc.tensor.matmul(out_ps[:], xnjT_sb[:], wnT_sb[:], start=False, stop=False)
                # the e-mean: 8 accumulating matmuls against weT2 (the
                # parity-stacked We.T/K); PSUM f32 accumulation
                for kk in range(K // 2):
                    nc.tensor.matmul(
                        out_ps[:],
                        e_sb[:, e_off + kk * P:e_off + (kk + 1) * P],
                        weT2_sb[:],
                        start=False, stop=(kk == K // 2 - 1),
                    )

                # ReLU (+cast to bf16) into the chunk staging buffer, on ACT
                nc.scalar.activation(
                    o_stage[:, i * XN_OUT:(i + 1) * XN_OUT], out_ps[:],
                    mybir.ActivationFunctionType.Relu,
                )
                t += 1

            nc.sync.dma_start(outs[c][:, :], o_stage[:])

    from concourse.library_overlay import lower_extended_insts

    lower_extended_insts(nc)
    _legalize_waits(nc, nop_sem)
    return nc


def _legalize_waits(nc: bass.Bass, nop_sem) -> None:
    """Split multi-wait queue-DMAs / matmuls for walrus's 1-wait codegen limit.

    The TRN2 walrus codegen allows a single sync-wait command per queue-DMA
    entry and per PE matmul (S3_LW struct). Tile emits minimal waits but can
    still produce 2+ (e.g. a slot's previous-writer DMA completion plus its
    last-reader engine release - Tile's clocks are not transitive). Queue
    entries execute in FIFO order, so extra waits are moved onto tiny no-op
    carrier DMAs inserted immediately before the offender on the same queue.
    For matmuls the carrier is a 1-column bf16 LDWEIGHTS (any clobbered
    weights are reloaded by each matmul's own weight load; insertion happens
    before a directly-preceding LDWEIGHTS so split LDW+MM pairs stay intact).
    """
    import copy

    dma_tpl: dict = {}
    eng_tpl: dict = {}
    evsem_tpl: dict = {}
    ldw_tpl = None
    for f in nc.m.functions:
        for blk in f.blocks:
            for inst in blk.instructions:
                tn = type(inst).__name__
                dst = (
                    str(getattr(inst.outs[0], "memref", "")) if inst.outs else ""
                )
                if tn == "InstDMACopy":
                    if dst.startswith("nop_hw"):
                        dma_tpl["qSPDynamicHW"] = inst
                    elif dst.startswith("nop_sw"):
                        dma_tpl[inst.queue] = inst
                elif tn == "InstLdweights" and ldw_tpl is None:
                    ldw_tpl = inst
                elif tn == "InstEventSemaphore":
                    evsem_tpl[inst.engine] = inst
                elif dst.startswith("nop_dve") or dst.startswith("nop_act") or dst.startswith("nop_pool"):
                    eng_tpl[inst.engine] = inst

    counter = [0]

    def make_nop(tpl, wait):
        counter[0] += 1
        nop = copy.deepcopy(tpl)
        nop.name = f"I-{nc.next_id()}"
        # DMA carriers must update a semaphore (BIR invariant); use a
        # dedicated one nobody waits on. Other engines' carriers stay
        # update-free (walrus rejects a waitnop update on e.g. TensorCopy
        # with a no_semaphore_value_conflict ISA check).
        upd = []
        if type(tpl).__name__ == "InstDMACopy":
            upd = [
                mybir.SyncUpdate(
                    sync_type="semaphore",
                    id=nop_sem.num,
                    ant_name=nop_sem.name,
                    update_mode="sem-add-imm",
                    update_value=16,
                )
            ]
        nop.sync_info = mybir.SyncInfo(on_wait=[wait], on_update=upd)
        nc.inst_map[nop.name] = nop
        return nop

    for f in nc.m.functions:
        for blk in f.blocks:
            out: list = []
            changed = False
            insts = list(blk.instructions)
            for pos, inst in enumerate(insts):
                tn = type(inst).__name__
                si = inst.sync_info
                waits = list(si.on_wait) if si else []
                nops = None
                if len(waits) > 1:
                    if tn == "InstDMACopy":
                        tpl = dma_tpl.get(inst.queue)
                        assert tpl is not None, f"no nop template for {inst.queue}"
                        nops = [make_nop(tpl, w) for w in waits[:-1]]
                    elif tn in ("InstMatmult", "InstLdweights"):
                        assert ldw_tpl is not None, "no ldweights template"
                        nops = [make_nop(ldw_tpl, w) for w in waits[:-1]]
                        # keep split LDW+MM pairs adjacent
                        if out and type(out[-1]).__name__ == "InstLdweights":
                            own_ldw = out.pop()
                            nops.append(own_ldw)
                    elif tn == "InstDrain":
                        # a drain is its own carrier: extra single-wait drains
                        # on the same engine are harmless
                        nops = [make_nop(inst, w) for w in waits[:-1]]
                    elif inst.engine in eng_tpl and tn not in (
                        "InstDrain",
                        "InstEventSemaphore",
                        "InstSemaphoreOp",
                    ):
                        nops = [make_nop(eng_tpl[inst.engine], w) for w in waits[:-1]]
                if nops:
                    out.extend(nops)
                    inst.sync_info = mybir.SyncInfo(
                        on_wait=waits[-1:], on_update=list(si.on_update)
                    )
                    changed = True
                out.append(inst)
            if changed:
                try:
                    blk.instructions[:] = out
                except TypeError:
                    blk.instructions.clear()
                    blk.instructions.extend(out)


_PROGRAM_CACHE: dict = {}


def _get_program(n_loc_pad: int, n_src: int) -> bass.Bass:
    key = (n_loc_pad, n_src)
    if key not in _PROGRAM_CACHE:
        _PROGRAM_CACHE[key] = build_program(n_loc_pad, n_src)
    return _PROGRAM_CACHE[key]


def prep_gather(nh_pad: np.ndarray, grp: int):
    """Bucket edges by nh%4 per tile, emit int16 super-row ids (wrapped
    [16, L/16] layout replicated to 128 partitions) and per-slot node ids.

    Returns (idx16 [128, n_groups*NCLS*seg_i16], nid [128, n_tiles*CH_T] i16).
    """
    n_pad = nh_pad.shape[0]
    n_tiles = n_pad // P
    n_groups = n_tiles // grp
    seg_i16 = grp * SEG // 16

    idx16 = np.zeros((n_groups * NCLS, grp * SEG), np.int16)
    nid = np.full((n_tiles, CH_T, P), 255, np.int16)  # [tile, chunk, slot%128]
    nodes_tpl = np.repeat(np.arange(P, dtype=np.int16), K)  # edge -> node
    for t in range(n_tiles):
        nh_t = nh_pad[t * P:(t + 1) * P]          # [128 nodes, K]
        vals = nh_t.reshape(-1)                    # edge -> neighbor id
        cls = vals % NCLS
        g, ti = divmod(t, grp)
        for j in range(NCLS):
            sel = np.nonzero(cls == j)[0]
            l = len(sel)
            assert l <= SEG, f"class overflow {l} > {SEG}"
            idx16[g * NCLS + j, ti * SEG:ti * SEG + l] = (
                vals[sel] // NCLS
            ).astype(np.int16)
            # local slot s of class j -> tile chunk j*NCH + s//128, row s%128
            nid[t, j * NCH:j * NCH + (l + P - 1) // P].reshape(-1)[:l] = (
                nodes_tpl[sel]
            )
    # wrap idx16: entry i -> [i%16, i//16]; replicate 16-row block to 128
    idx16 = idx16.reshape(n_groups * NCLS, grp * SEG // 16, 16).transpose(0, 2, 1)
    idx16 = np.tile(idx16, (1, 8, 1)).reshape(n_groups, NCLS, P, seg_i16)
    idx16 = np.ascontiguousarray(
        idx16.transpose(2, 0, 1, 3).reshape(P, n_groups * NCLS * seg_i16)
    )
    # nid: [tile, chunk, slot] -> [slot(part), tile*CH_T + chunk]
    nid = np.ascontiguousarray(
        nid.transpose(2, 0, 1).reshape(P, n_tiles * CH_T)
    )
    return idx16, nid


def assemble_out(res_core: dict, n_tiles: int) -> np.ndarray:
    """Per-chunk partition-major bf16 outputs -> [n_loc_pad, 128] f32."""
    parts = []
    for c, ct in enumerate(_chunks(n_tiles)):
        o = np.asarray(res_core[f"out{c}"]).astype(np.float32)  # [128, ct*128]
        parts.append(
            o.reshape(P, ct, XN_OUT).transpose(1, 0, 2).reshape(ct * P, XN_OUT)
        )
    return np.concatenate(parts, axis=0)


def make_in_maps(x, e, ij, Wc, Wn, We, n_cores=N_CORES):
    """Host-side shard/prep: per-core input dicts for the SPMD program."""
    import ml_dtypes

    bf16 = ml_dtypes.bfloat16
    n = x.shape[0]
    n_loc = n // n_cores
    n_loc_pad = ((n_loc + P - 1) // P) * P
    n_tiles = n_loc_pad // P
    grp = GRP if n_tiles % GRP == 0 else (2 if n_tiles % 2 == 0 else 1)

    x = np.asarray(x, np.float32)
    assert n % NCLS == 0
    x4 = np.ascontiguousarray(x.reshape(n // NCLS, NCLS * XN_IN)).astype(bf16)
    nh = np.ascontiguousarray(ij[:, :, 0]).astype(np.int32)
    wcT = np.ascontiguousarray(Wc.T).astype(bf16)
    wnT = (np.ascontiguousarray(Wn.T) / np.float32(K)).astype(bf16)
    # weT2[(par*64+f), o] = We[o, f] / K   (both parity halves identical)
    weT2 = np.tile(np.ascontiguousarray(We.T) / np.float32(K), (2, 1)).astype(bf16)

    in_maps = []
    for c in range(n_cores):
        sl = slice(c * n_loc, (c + 1) * n_loc)
        xs = np.zeros((n_loc_pad, XN_IN), np.float32)
        xs[:n_loc] = x[sl]
        xT = np.ascontiguousarray(xs.T).astype(bf16)  # [128 f, n_loc_pad]
        # eT[(par*64+f), t, kk, n] = e[t*128+n, 2kk+par, f]
        e_c = np.zeros((n_loc_pad, K, XE_IN), np.float32)
        e_c[:n_loc] = np.asarray(e[sl], np.float32)
        eT = e_c.reshape(n_tiles, P, K // 2, 2, XE_IN)  # [t, n, kk, par, f]
        eT = np.ascontiguousarray(
            eT.transpose(3, 4, 0, 2, 1)  # [par, f, t, kk, n]
        ).reshape(P, n_tiles * (K // 2) * P).astype(bf16)
        # pad rows cycle 0..3 so no per-tile mod-class bucket overflows SEG
        idx_c = np.tile(np.arange(K, dtype=np.int32) % NCLS, (n_loc_pad, 1))
        idx_c[:n_loc] = nh[sl]
        idx16, nid = prep_gather(idx_c, grp)
        in_maps.append(
            {
                "x4": x4,
                "xT_loc": xT,
                "eT_loc": eT,
                "idx_loc": idx16,
                "nid_loc": nid,
                "wcT": wcT,
                "wnT": wnT,
                "weT2": weT2,
            }
        )
    return in_maps, n_loc, n_loc_pad


def kernel(x, e, ij, Wc, Wn, We):
    x = np.asarray(x)
    e = np.asarray(e)
    ij = np.asarray(ij)
    in_maps, n_loc, n_loc_pad = make_in_maps(x, e, ij, Wc, Wn, We)
    nc = _get_program(n_loc_pad, x.shape[0])
    res = run_bass_kernel_spmd(nc, in_maps, list(range(N_CORES)))
    n_tiles = n_loc_pad // P
    out = np.concatenate(
        [assemble_out(r, n_tiles)[:n_loc] for r in res.results], axis=0
    )
    return out.astype(np.float32)


# revision 40
# speedup vs baseline: 1.9056x; 1.9056x over previous
"""Trainium2 Bass kernel: GNN conv block (nn_Conv_block_49331994362308).

Computes, for N=100000 nodes with K=16 neighbors each:
    nh  = ij[:, :, 0]                      # [N, K] neighbor ids
    xnj = mean(x[nh], axis=1)              # neighbor-feature mean  [N, 128]
    xej = mean(e, axis=1)                  # edge-feature mean      [N, 64]
    out = relu(x @ Wc.T + xnj @ Wn.T + xej @ We.T)

Distribution: data-parallel over nodes across 8 NeuronCores (12500 nodes
per core, padded to 12544 = 98*128). x is replicated to every core (bf16)
so the random neighbor gather x[nh] is a core-local indirect DMA from HBM.

v2 (vs the f32 baseline):
  - Everything device-side is bf16 (host pre-casts; 1/K folded into the
    weights; output bf16, host casts back to f32). Halves e/x/out DMA.
  - The gather pulls bf16 rows (256B elems) directly - no ACT cast.
  - Gather groups of GRP=7 tiles: 4 SWDGE instructions per 7 tiles
    (fixed ~1us Q7 cost per instruction dominates, so fewer is better).
  - No PE transposes: x ships pre-transposed (xT_loc), e ships in a
    (parity*feature, kk, node) layout whose DVE reduce directly yields
    the final-matmul lhsT, with weT2 = [We.T; We.T]/K contracting both
    parity halves; xnj pooling already yields [feature, node].
  - One-hot pooling matrices are built ON DEVICE: host ships per-slot
    node ids (int16, 255=pad), one DVE is_equal per tile expands them
    against a constant iota to the fp8 [slot, node] one-hot.

Per-core pipeline per 128-node tile:
  Pool: (per 7-tile group) 4x dma_gather of class nh%4 super-rows.
  DVE:  is_equal one-hot build; e-mean via strided tensor_reduce.
  PE:   20 accumulating bf16xfp8 pool matmuls (xnjT = sum x[nh].T),
        then 3 accumulating bf16 matmuls against the weights.
  ACT:  xnjT PSUM->bf16 cast; ReLU into the per-14-tile staging buffer.

Walrus's TRN2 queue-DMA codegen only supports ONE sync-wait command per
DMA (and one per PE LDWEIGHTS), so the structure keeps every DMA at a
single dependency front: indices/nodeids are preloaded once into SBUF,
the 8 SWDGE bookkeeping lanes are warmed with dummy transfers that
absorb the preload front, and outputs go to once-written per-chunk DRAM
tensors (no WAW chains). _legalize_waits moves any residual extra waits
onto no-op carrier instructions.
"""

from contextlib import ExitStack

import numpy as np

import concourse.bass as bass
import concourse.mybir as mybir
import concourse.tile as tile
from concourse.bass_utils import run_bass_kernel_spmd
from concourse import library_config

P = 128
K = 16
XN_IN = 128
XE_IN = 64
XN_OUT = 128
N_CORES = 8
N_FULL = 100000
N_LOC = N_FULL // N_CORES          # 12500
N_LOC_PAD = ((N_LOC + P - 1) // P) * P  # 12544
CHUNK = 14                          # tiles per output chunk (98 = 7*14)

F32 = mybir.dt.float32
BF16 = mybir.dt.bfloat16
F8 = mybir.dt.float8e4   # one-hot pooling matrices hold only 0/1 - exact
I16 = mybir.dt.int16

GRP = 7            # tiles per gather group (must divide n_tiles)
NCLS = 4           # x rows per int16 "super-row" (mod classes)
SEG = 640          # padded gather slots per (tile, class); 5 chunks of 128
NCH = SEG // P     # chunks per (tile, class) = 5
CH_T = NCH * NCLS  # pool chunks per tile = 20


def _chunks(n_tiles: int) -> list[int]:
    out = []
    t = 0
    while t < n_tiles:
        out.append(min(CHUNK, n_tiles - t))
        t += CHUNK
    return out


def build_program(n_loc_pad: int, n_src: int) -> bass.Bass:
    """Build the SPMD per-core Bass program (same program on every core)."""
    assert n_loc_pad % P == 0
    n_tiles = n_loc_pad // P
    chunks = _chunks(n_tiles)

    # detect_race_conditions=False: the post-schedule wait-legalizer's nop
    # carriers share scratch tiles and trip the sim race detector's
    # bookkeeping (same-engine program order makes them safe).
    nc = bass.Bass("TRN2", debug=False, detect_race_conditions=False)

    grp = GRP if n_tiles % GRP == 0 else (2 if n_tiles % 2 == 0 else 1)
    n_groups = n_tiles // grp
    seg_i16 = grp * SEG // 16  # idx16 columns per (group, class)

    x4 = nc.dram_tensor("x4", [n_src // NCLS, NCLS * XN_IN], BF16,
                        kind="ExternalInput").ap()
    xT_loc = nc.dram_tensor("xT_loc", [P, n_tiles * P], BF16,
                            kind="ExternalInput").ap()
    eT_loc = nc.dram_tensor("eT_loc", [P, n_tiles * (K // 2) * P], BF16,
                            kind="ExternalInput").ap()
    # int16 super-row ids (nh//4), wrapped [16, L/16] + replicated to 128
    # partitions, concatenated over (group, class)
    idx_loc = nc.dram_tensor(
        "idx_loc", [P, n_groups * NCLS * seg_i16], I16, kind="ExternalInput"
    ).ap()
    # per-slot node ids (0..127, 255=pad): [128 slot, tile*CH_T chunks]
    nid_loc = nc.dram_tensor(
        "nid_loc", [P, n_tiles * CH_T], I16, kind="ExternalInput"
    ).ap()
    wcT = nc.dram_tensor("wcT", [XN_IN, XN_OUT], BF16, kind="ExternalInput").ap()
    wnT = nc.dram_tensor("wnT", [XN_IN, XN_OUT], BF16, kind="ExternalInput").ap()
    weT2 = nc.dram_tensor("weT2", [P, XN_OUT], BF16, kind="ExternalInput").ap()
    # per-chunk outputs, partition-major: out_c[p, i*128+f] = out[(t0+i)*128+p, f]
    outs = [
        nc.dram_tensor(f"out{c}", [P, ct * XN_OUT], BF16, kind="ExternalOutput").ap()
        for c, ct in enumerate(chunks)
    ]

    nop_sem = nc.alloc_semaphore("waitnop")

    with tile.TileContext(nc) as tc, ExitStack() as ctx:
        nc.gpsimd.sem_clear(range(nop_sem.num, nop_sem.num + 1))
        consts = ctx.enter_context(tc.tile_pool(name="consts", bufs=1))
        # iota_t[p, n*CH_T+c] = n  (for the is_equal one-hot expansion; the
        # [slot, node, chunk] layout keeps the nid broadcast OFF the last
        # dim, which the DVE runs at 2x — measured 1.49us vs 2.81us/tile).
        # Emitted before load_library: Iota lives in the 'standard' Pool
        # library, dma_gather in 'mlp'.
        iota_t = consts.tile([P, P * CH_T], I16, tag="iota_t")
        nc.gpsimd.iota(
            iota_t[:].rearrange("p (n c) -> p n c", c=CH_T),
            pattern=[[1, P], [0, CH_T]],
            channel_multiplier=0,
        )
        nc.gpsimd.load_library(library_config.mlp)
        wcT_sb = consts.tile([XN_IN, XN_OUT], BF16, tag="wc")
        wnT_sb = consts.tile([XN_IN, XN_OUT], BF16, tag="wn")
        weT2_sb = consts.tile([P, XN_OUT], BF16, tag="we2")
        nc.sync.dma_start(wcT_sb[:], wcT[:, :])
        nc.sync.dma_start(wnT_sb[:], wnT[:, :])
        nc.sync.dma_start(weT2_sb[:], weT2[:, :])
        nid_all = consts.tile([P, n_tiles * CH_T], I16, tag="nid_all")
        nc.sync.dma_start(nid_all[:], nid_loc[:, :])
        # x viewed as [n_src/4, 4, 128] bf16: class j gathers row 4*i16+j via
        # elem_step=512 elements (1024B stride) and a j*128-element offset
        x4v = x4.rearrange("r (c f) -> r c f", c=NCLS)

        # Warm the 8 SWDGE bookkeeping lanes: each dummy absorbs the
        # idx-preload front so later gathers carry only their PE front.
        scratch = ctx.enter_context(tc.tile_pool(name="scratch", bufs=1))
        for q in range(8):
            sc = scratch.tile([1, K], I16, tag=f"sc{q}")
            nc.gpsimd.dma_start(sc[:], iota_t[:1, :K])
        # Tiny template instructions for _legalize_waits nop carriers
        # (one per DMA queue and per compute engine).
        nop_hw = scratch.tile([1, K], I16, tag="noptpl_hw")
        nc.sync.dma_start(nop_hw[:], idx_loc[:1, :K])
        nop_hwa = scratch.tile([1, K], I16, tag="noptpl_hwa")
        nc.scalar.dma_start(nop_hwa[:], idx_loc[:1, :K])
        nop_sw = scratch.tile([1, K], I16, tag="noptpl_sw")
        nc.gpsimd.dma_start(nop_sw[:], idx_loc[:1, :K])
        nop_dve = scratch.tile([P, K], I16, tag="noptpl_dve")
        nc.vector.tensor_copy(nop_dve[:], iota_t[:, :K])
        nop_act = scratch.tile([P, K], I16, tag="noptpl_act")
        nc.scalar.copy(nop_act[:], iota_t[:, :K])
        nop_pool = scratch.tile([P, K], F32, tag="noptpl_pool")
        nc.gpsimd.memset(nop_pool[:], 0.0)

        g_pool = ctx.enter_context(tc.tile_pool(name="gatherp", bufs=3))
        idx_pool = ctx.enter_context(tc.tile_pool(name="idxp", bufs=3))
        pp_pool = ctx.enter_context(tc.tile_pool(name="poolmat", bufs=2))
        e_pool = ctx.enter_context(tc.tile_pool(name="edgep", bufs=3))
        xs_pool = ctx.enter_context(tc.tile_pool(name="xselfp", bufs=2))
        st_pool = ctx.enter_context(tc.tile_pool(name="stagep", bufs=3))
        out_pool = ctx.enter_context(tc.tile_pool(name="outp", bufs=2))
        psum_pool = ctx.enter_context(tc.tile_pool(name="psump", bufs=2, space="PSUM"))
        psum1_pool = ctx.enter_context(tc.tile_pool(name="psum1p", bufs=2, space="PSUM"))

        # Warm up PE's view of the constant weights so steady-state matmuls
        # carry at most one sync wait (PE LDWEIGHTS supports a single wait).
        ps_warm = psum1_pool.tile([P, P], F32, tag="warm")
        nc.tensor.matmul(ps_warm[:], wcT_sb[:], wcT_sb[:], start=True, stop=False)
        nc.tensor.matmul(ps_warm[:], wnT_sb[:], wnT_sb[:], start=False, stop=False)
        nc.tensor.matmul(ps_warm[:], weT2_sb[:], weT2_sb[:], start=False, stop=True)

        t = 0
        gbf = [None] * NCLS
        nidx_reg = nc.gpsimd.to_reg(grp * SEG)  # shared across all gathers
        for c, ct in enumerate(chunks):
            o_stage = out_pool.tile([P, ct * XN_OUT], BF16, tag="ostage")
            for i in range(ct):
                g, ti = divmod(t, grp)

                if ti == 0:
                    # per-group idx streams, double-buffered just-in-time
                    idx_g = idx_pool.tile([P, NCLS * seg_i16], I16, tag="idxg")
                    nc.sync.dma_start(
                        idx_g[:],
                        idx_loc[:, g * NCLS * seg_i16:(g + 1) * NCLS * seg_i16],
                    )
                    # per-group gathers: one dma_gather per mod-4 class of
                    # grp*SEG slots; slot s lands at partition s%128, free
                    # block s//128, so 128-slot chunks stay within one tile.
                    for j in range(NCLS):
                        gb = g_pool.tile(
                            [P, grp * SEG // P, XN_IN], BF16, tag=f"gb{j}"
                        )
                        nc.gpsimd.dma_gather(
                            out_ap=gb[:],
                            in_ap=x4v[:, j, :],
                            idxs_ap=idx_g[:, j * seg_i16:(j + 1) * seg_i16],
                            num_idxs=grp * SEG,
                            num_idxs_reg=nidx_reg,
                            elem_size=XN_IN,
                            elem_step=NCLS * XN_IN,
                            single_packet=False,
                        )
                        gbf[j] = gb[:].rearrange("p b f -> p (b f)")

                # one-hot P[slot, n*CH_T+c] = (nid[slot, c] == n), bf16;
                # node-major layout so the nid broadcast is on the middle dim
                p_sb = pp_pool.tile([P, P * CH_T], BF16, tag="pmat")
                nid_b = nid_all[:, t * CH_T:(t + 1) * CH_T]
                in0, in1 = bass.broadcast_tensor_aps(
                    iota_t[:].rearrange("p (n c) -> p n c", c=CH_T),
                    nid_b.rearrange("p (one c) -> p one c", one=1),
                )
                nc.vector.tensor_tensor(
                    p_sb[:].rearrange("p (n c) -> p n c", c=CH_T),
                    in0, in1, op=mybir.AluOpType.is_equal,
                )
                p_view = p_sb[:].rearrange("p (n c) -> p c n", c=CH_T)

                # e arrives pre-permuted: e_sb[p=(par,f), kk*128+n]; the mean
                # over k happens on PE as 8 extra accumulating matmuls below.
                # Two tiles per DMA, issued on the ACT HWDGE queue to keep
                # the SP sequencer off the critical path.
                if ti == 0:
                    xT_sb = xs_pool.tile([P, grp * P], BF16, tag="xT")
                    nc.sync.dma_start(
                        xT_sb[:], xT_loc[:, t * P:(t + grp) * P]
                    )
                if t % 2 == 0:
                    e_sb = e_pool.tile([P, 2 * (K // 2) * P], BF16, tag="e")
                    nc.sync.dma_start(
                        e_sb[:],
                        eT_loc[:, t * (K // 2) * P:(t + 2) * (K // 2) * P],
                    )
                e_off = (t % 2) * (K // 2) * P

                # xnjT[f, n] = sum_slot g[slot, f] * P[slot, n]
                xnjT_ps = psum_pool.tile([P, P], F32, tag="ps_xnj")
                for b in range(CH_T):
                    j, bl = divmod(b, NCH)
                    blk = ti * NCH + bl
                    nc.tensor.matmul(
                        xnjT_ps[:],
                        gbf[j][:, blk * XN_IN:(blk + 1) * XN_IN],
                        p_view[:, b, :],
                        start=(b == 0),
                        stop=(b == CH_T - 1),
                    )
                xnjT_sb = st_pool.tile([P, P], BF16, tag="sb_xnj")
                nc.scalar.copy(xnjT_sb[:], xnjT_ps[:])

                out_ps = psum1_pool.tile([P, XN_OUT], F32, tag="ps_out")
                nc.tensor.matmul(
                    out_ps[:], xT_sb[:, ti * P:(ti + 1) * P], wcT_sb[:],
                    start=True, stop=False,
                )
                nc.tensor.matmul(out_ps[:], xnjT_sb[:], wnT_sb[:], start=False, stop=False)
                # the e-mean: 8 accumulating matmuls against weT2 (the
                # parity-stacked We.T/K); PSUM f32 accumulation
                for kk in range(K // 2):
                    nc.tensor.matmul(
                        out_ps[:],
                        e_sb[:, e_off + kk * P:e_off + (kk + 1) * P],
                        weT2_sb[:],
                        start=False, stop=(kk == K // 2 - 1),
                    )

                # ReLU (+cast to bf16) into the chunk staging buffer, on ACT
                nc.scalar.activation(
                    o_stage[:, i * XN_OUT:(i + 1) * XN_OUT], out_ps[:],
                    mybir.ActivationFunctionType.Relu,
                )
                t += 1

            nc.sync.dma_start(outs[c][:, :], o_stage[:])

    from concourse.library_overlay import lower_extended_insts

    lower_extended_insts(nc)
    _legalize_waits(nc, nop_sem)
    return nc


def _legalize_waits(nc: bass.Bass, nop_sem) -> None:
    """Split multi-wait queue-DMAs / matmuls for walrus's 1-wait codegen limit.

    The TRN2 walrus codegen allows a single sync-wait command per queue-DMA
    entry and per PE matmul (S3_LW struct). Tile emits minimal waits but can
    still produce 2+ (e.g. a slot's previous-writer DMA completion plus its
    last-reader engine release - Tile's clocks are not transitive). Queue
    entries execute in FIFO order, so extra waits are moved onto tiny no-op
    carrier DMAs inserted immediately before the offender on the same queue.
    For matmuls the carrier is a 1-column bf16 LDWEIGHTS (any clobbered
    weights are reloaded by each matmul's own weight load; insertion happens
    before a directly-preceding LDWEIGHTS so split LDW+MM pairs stay intact).
    """
    import copy

    dma_tpl: dict = {}
    eng_tpl: dict = {}
    evsem_tpl: dict = {}
    ldw_tpl = None
    for f in nc.m.functions:
        for blk in f.blocks:
            for inst in blk.instructions:
                tn = type(inst).__name__
                dst = (
                    str(getattr(inst.outs[0], "memref", "")) if inst.outs else ""
                )
                if tn == "InstDMACopy":
                    if dst.startswith("nop_hw") or dst.startswith("nop_sw"):
                        dma_tpl[inst.queue] = inst
                elif tn == "InstLdweights" and ldw_tpl is None:
                    ldw_tpl = inst
                elif tn == "InstEventSemaphore":
                    evsem_tpl[inst.engine] = inst
                elif dst.startswith("nop_dve") or dst.startswith("nop_act") or dst.startswith("nop_pool"):
                    eng_tpl[inst.engine] = inst

    counter = [0]

    def make_nop(tpl, wait):
        counter[0] += 1
        nop = copy.deepcopy(tpl)
        nop.name = f"I-{nc.next_id()}"
        # DMA carriers must update a semaphore (BIR invariant); use a
        # dedicated one nobody waits on. Other engines' carriers stay
        # update-free (walrus rejects a waitnop update on e.g. TensorCopy
        # with a no_semaphore_value_conflict ISA check).
        upd = []
        if type(tpl).__name__ == "InstDMACopy":
            upd = [
                mybir.SyncUpdate(
                    sync_type="semaphore",
                    id=nop_sem.num,
                    ant_name=nop_sem.name,
                    update_mode="sem-add-imm",
                    update_value=16,
                )
            ]
        nop.sync_info = mybir.SyncInfo(on_wait=[wait], on_update=upd)
        nc.inst_map[nop.name] = nop
        return nop

    for f in nc.m.functions:
        for blk in f.blocks:
            out: list = []
            changed = False
            insts = list(blk.instructions)
            for pos, inst in enumerate(insts):
                tn = type(inst).__name__
                si = inst.sync_info
                waits = list(si.on_wait) if si else []
                nops = None
                if len(waits) > 1:
                    if tn == "InstDMACopy":
                        tpl = dma_tpl.get(inst.queue)
                        assert tpl is not None, f"no nop template for {inst.queue}"
                        nops = [make_nop(tpl, w) for w in waits[:-1]]
                    elif tn in ("InstMatmult", "InstLdweights"):
                        assert ldw_tpl is not None, "no ldweights template"
                        nops = [make_nop(ldw_tpl, w) for w in waits[:-1]]
                        # keep split LDW+MM pairs adjacent
                        if out and type(out[-1]).__name__ == "InstLdweights":
                            own_ldw = out.pop()
                            nops.append(own_ldw)
                    elif tn == "InstDrain":
                        # a drain is its own carrier: extra single-wait drains
                        # on the same engine are harmless
                        nops = [make_nop(inst, w) for w in waits[:-1]]
                    elif inst.engine in eng_tpl and tn not in (
                        "InstDrain",
                        "InstEventSemaphore",
                        "InstSemaphoreOp",
                    ):
                        nops = [make_nop(eng_tpl[inst.engine], w) for w in waits[:-1]]
                if nops:
                    out.extend(nops)
                    inst.sync_info = mybir.SyncInfo(
                        on_wait=waits[-1:], on_update=list(si.on_update)
                    )
                    changed = True
                out.append(inst)
            if changed:
                try:
                    blk.instructions[:] = out
                except TypeError:
                    blk.instructions.clear()
                    blk.instructions.extend(out)


_PROGRAM_CACHE: dict = {}


def _get_program(n_loc_pad: int, n_src: int) -> bass.Bass:
    key = (n_loc_pad, n_src)
    if key not in _PROGRAM_CACHE:
        _PROGRAM_CACHE[key] = build_program(n_loc_pad, n_src)
    return _PROGRAM_CACHE[key]


def prep_gather(nh_pad: np.ndarray, grp: int):
    """Bucket edges by nh%4 per tile, emit int16 super-row ids (wrapped
    [16, L/16] layout replicated to 128 partitions) and per-slot node ids.

    Returns (idx16 [128, n_groups*NCLS*seg_i16], nid [128, n_tiles*CH_T] i16).
    """
    n_pad = nh_pad.shape[0]
    n_tiles = n_pad // P
    n_groups = n_tiles // grp
    seg_i16 = grp * SEG // 16

    idx16 = np.zeros((n_groups * NCLS, grp * SEG), np.int16)
    nid = np.full((n_tiles, CH_T, P), 255, np.int16)  # [tile, chunk, slot%128]
    nodes_tpl = np.repeat(np.arange(P, dtype=np.int16), K)  # edge -> node
    for t in range(n_tiles):
        nh_t = nh_pad[t * P:(t + 1) * P]          # [128 nodes, K]
        vals = nh_t.reshape(-1)                    # edge -> neighbor id
        cls = vals % NCLS
        g, ti = divmod(t, grp)
        for j in range(NCLS):
            sel = np.nonzero(cls == j)[0]
            l = len(sel)
            assert l <= SEG, f"class overflow {l} > {SEG}"
            idx16[g * NCLS + j, ti * SEG:ti * SEG + l] = (
                vals[sel] // NCLS
            ).astype(np.int16)
            # local slot s of class j -> tile chunk j*NCH + s//128, row s%128
            nid[t, j * NCH:j * NCH + (l + P - 1) // P].reshape(-1)[:l] = (
                nodes_tpl[sel]
            )
    # wrap idx16: entry i -> [i%16, i//16]; replicate 16-row block to 128
    idx16 = idx16.reshape(n_groups * NCLS, grp * SEG // 16, 16).transpose(0, 2, 1)
    idx16 = np.tile(idx16, (1, 8, 1)).reshape(n_groups, NCLS, P, seg_i16)
    idx16 = np.ascontiguousarray(
        idx16.transpose(2, 0, 1, 3).reshape(P, n_groups * NCLS * seg_i16)
    )
    # nid: [tile, chunk, slot] -> [slot(part), tile*CH_T + chunk]
    nid = np.ascontiguousarray(
        nid.transpose(2, 0, 1).reshape(P, n_tiles * CH_T)
    )
    return idx16, nid


def assemble_out(res_core: dict, n_tiles: int) -> np.ndarray:
    """Per-chunk partition-major bf16 outputs -> [n_loc_pad, 128] f32."""
    parts = []
    for c, ct in enumerate(_chunks(n_tiles)):
        o = np.asarray(res_core[f"out{c}"]).astype(np.float32)  # [128, ct*128]
        parts.append(
            o.reshape(P, ct, XN_OUT).transpose(1, 0, 2).reshape(ct * P, XN_OUT)
        )
    return np.concatenate(parts, axis=0)


def make_in_maps(x, e, ij, Wc, Wn, We, n_cores=N_CORES):
    """Host-side shard/prep: per-core input dicts for the SPMD program."""
    import ml_dtypes

    bf16 = ml_dtypes.bfloat16
    n = x.shape[0]
    n_loc = n // n_cores
    n_loc_pad = ((n_loc + P - 1) // P) * P
    n_tiles = n_loc_pad // P
    grp = GRP if n_tiles % GRP == 0 else (2 if n_tiles % 2 == 0 else 1)

    x = np.asarray(x, np.float32)
    assert n % NCLS == 0
    x4 = np.ascontiguousarray(x.reshape(n // NCLS, NCLS * XN_IN)).astype(bf16)
    nh = np.ascontiguousarray(ij[:, :, 0]).astype(np.int32)
    wcT = np.ascontiguousarray(Wc.T).astype(bf16)
    wnT = (np.ascontiguousarray(Wn.T) / np.float32(K)).astype(bf16)
    # weT2[(par*64+f), o] = We[o, f] / K   (both parity halves identical)
    weT2 = np.tile(np.ascontiguousarray(We.T) / np.float32(K), (2, 1)).astype(bf16)

    in_maps = []
    for c in range(n_cores):
        sl = slice(c * n_loc, (c + 1) * n_loc)
        xs = np.zeros((n_loc_pad, XN_IN), np.float32)
        xs[:n_loc] = x[sl]
        xT = np.ascontiguousarray(xs.T).astype(bf16)  # [128 f, n_loc_pad]
        # eT[(par*64+f), t, kk, n] = e[t*128+n, 2kk+par, f]
        e_c = np.zeros((n_loc_pad, K, XE_IN), np.float32)
        e_c[:n_loc] = np.asarray(e[sl], np.float32)
        eT = e_c.reshape(n_tiles, P, K // 2, 2, XE_IN)  # [t, n, kk, par, f]
        eT = np.ascontiguousarray(
            eT.transpose(3, 4, 0, 2, 1)  # [par, f, t, kk, n]
        ).reshape(P, n_tiles * (K // 2) * P).astype(bf16)
        # pad rows cycle 0..3 so no per-tile mod-class bucket overflows SEG
        idx_c = np.tile(np.arange(K, dtype=np.int32) % NCLS, (n_loc_pad, 1))
        idx_c[:n_loc] = nh[sl]
        idx16, nid = prep_gather(idx_c, grp)
        in_maps.append(
            {
                "x4": x4,
                "xT_loc": xT,
                "eT_loc": eT,
                "idx_loc": idx16,
                "nid_loc": nid,
                "wcT": wcT,
                "wnT": wnT,
                "weT2": weT2,
            }
        )
    return in_maps, n_loc, n_loc_pad


def kernel(x, e, ij, Wc, Wn, We):
    x = np.asarray(x)
    e = np.asarray(e)
    ij = np.asarray(ij)
    in_maps, n_loc, n_loc_pad = make_in_maps(x, e, ij, Wc, Wn, We)
    nc = _get_program(n_loc_pad, x.shape[0])
    res = run_bass_kernel_spmd(nc, in_maps, list(range(N_CORES)))
    n_tiles = n_loc_pad // P
    out = np.concatenate(
        [assemble_out(r, n_tiles)[:n_loc] for r in res.results], axis=0
    )
    return out.astype(np.float32)


# revision 43
# speedup vs baseline: 4.0931x; 2.1479x over previous
"""Trainium2 Bass kernel: GNN conv block (nn_Conv_block_49331994362308).

Computes, for N=100000 nodes with K=16 neighbors each:
    nh  = ij[:, :, 0]                      # [N, K] neighbor ids
    xnj = mean(x[nh], axis=1)              # neighbor-feature mean  [N, 128]
    xej = mean(e, axis=1)                  # edge-feature mean      [N, 64]
    out = relu(x @ Wc.T + xnj @ Wn.T + xej @ We.T)

Distribution: data-parallel over nodes across 8 NeuronCores (12500 nodes
per core, padded to 12544 = 98*128). x is replicated to every core (bf16)
so the random neighbor gather x[nh] is a core-local indirect DMA from HBM.

v2 (vs the f32 baseline):
  - Everything device-side is bf16 (host pre-casts; 1/K folded into the
    weights; output bf16, host casts back to f32). Halves e/x/out DMA.
  - The gather pulls bf16 rows (256B elems) directly - no ACT cast.
  - Gather groups of GRP=7 tiles: 4 SWDGE instructions per 7 tiles
    (fixed ~1us Q7 cost per instruction dominates, so fewer is better).
  - No PE transposes: x ships pre-transposed (xT_loc), e ships in a
    (parity*feature, kk, node) layout whose DVE reduce directly yields
    the final-matmul lhsT, with weT2 = [We.T; We.T]/K contracting both
    parity halves; xnj pooling already yields [feature, node].
  - One-hot pooling matrices are built ON DEVICE: host ships per-slot
    node ids (int16, 255=pad), one DVE is_equal per tile expands them
    against a constant iota to the fp8 [slot, node] one-hot.

Per-core pipeline per 128-node tile:
  Pool: (per 7-tile group) 4x dma_gather of class nh%4 super-rows.
  DVE:  is_equal one-hot build; e-mean via strided tensor_reduce.
  PE:   20 accumulating bf16xfp8 pool matmuls (xnjT = sum x[nh].T),
        then 3 accumulating bf16 matmuls against the weights.
  ACT:  xnjT PSUM->bf16 cast; ReLU into the per-14-tile staging buffer.

Walrus's TRN2 queue-DMA codegen only supports ONE sync-wait command per
DMA (and one per PE LDWEIGHTS), so the structure keeps every DMA at a
single dependency front: indices/nodeids are preloaded once into SBUF,
the 8 SWDGE bookkeeping lanes are warmed with dummy transfers that
absorb the preload front, and outputs go to once-written per-chunk DRAM
tensors (no WAW chains). _legalize_waits moves any residual extra waits
onto no-op carrier instructions.
"""

from contextlib import ExitStack

import numpy as np

import concourse.bass as bass
import concourse.mybir as mybir
import concourse.tile as tile
from concourse.bass_utils import run_bass_kernel_spmd
from concourse import library_config

P = 128
K = 16
XN_IN = 128
XE_IN = 64
XN_OUT = 128
N_CORES = 8
N_FULL = 100000
N_LOC = N_FULL // N_CORES          # 12500
N_LOC_PAD = ((N_LOC + P - 1) // P) * P  # 12544
CHUNK = 14                          # tiles per output chunk (98 = 7*14)

F32 = mybir.dt.float32
BF16 = mybir.dt.bfloat16
F8 = mybir.dt.float8e4   # one-hot pooling matrices hold only 0/1 - exact
I16 = mybir.dt.int16

GRP = 7            # tiles per gather group (must divide n_tiles)
NCLS = 4           # x rows per int16 "super-row" (mod classes)
SEG = 640          # padded gather slots per (tile, class); 5 chunks of 128
NCH = SEG // P     # chunks per (tile, class) = 5
CH_T = NCH * NCLS  # pool chunks per tile = 20


def _chunks(n_tiles: int) -> list[int]:
    out = []
    t = 0
    while t < n_tiles:
        out.append(min(CHUNK, n_tiles - t))
        t += CHUNK
    return out


def build_program(n_loc_pad: int, n_src: int) -> bass.Bass:
    """Build the SPMD per-core Bass program (same program on every core)."""
    assert n_loc_pad % P == 0
    n_tiles = n_loc_pad // P
    chunks = _chunks(n_tiles)

    # detect_race_conditions=False: the post-schedule wait-legalizer's nop
    # carriers share scratch tiles and trip the sim race detector's
    # bookkeeping (same-engine program order makes them safe).
    nc = bass.Bass("TRN2", debug=False, detect_race_conditions=False)

    grp = GRP if n_tiles % GRP == 0 else (2 if n_tiles % 2 == 0 else 1)
    n_groups = n_tiles // grp
    seg_i16 = grp * SEG // 16  # idx16 columns per (group, class)

    x4 = nc.dram_tensor("x4", [n_src // NCLS, NCLS * XN_IN], BF16,
                        kind="ExternalInput").ap()
    xT_loc = nc.dram_tensor("xT_loc", [P, n_tiles * P], BF16,
                            kind="ExternalInput").ap()
    eT_loc = nc.dram_tensor("eT_loc", [P, n_tiles * (K // 2) * P], BF16,
                            kind="ExternalInput").ap()
    # int16 super-row ids (nh//4), wrapped [16, L/16] + replicated to 128
    # partitions, concatenated over (group, class)
    idx_loc = nc.dram_tensor(
        "idx_loc", [P, n_groups * NCLS * seg_i16], I16, kind="ExternalInput"
    ).ap()
    # per-slot node ids (0..127, 255=pad): [128 slot, tile*CH_T chunks]
    nid_loc = nc.dram_tensor(
        "nid_loc", [P, n_tiles * CH_T], I16, kind="ExternalInput"
    ).ap()
    wcT = nc.dram_tensor("wcT", [XN_IN, XN_OUT], BF16, kind="ExternalInput").ap()
    wnT = nc.dram_tensor("wnT", [XN_IN, XN_OUT], BF16, kind="ExternalInput").ap()
    weT2 = nc.dram_tensor("weT2", [P, XN_OUT], BF16, kind="ExternalInput").ap()
    # per-chunk outputs, partition-major: out_c[p, i*128+f] = out[(t0+i)*128+p, f]
    outs = [
        nc.dram_tensor(f"out{c}", [P, ct * XN_OUT], BF16, kind="ExternalOutput").ap()
        for c, ct in enumerate(chunks)
    ]

    nop_sem = nc.alloc_semaphore("waitnop")

    with tile.TileContext(nc) as tc, ExitStack() as ctx:
        nc.gpsimd.sem_clear(range(nop_sem.num, nop_sem.num + 1))
        consts = ctx.enter_context(tc.tile_pool(name="consts", bufs=1))
        # iota_t[p, n*CH_T+c] = n  (for the is_equal one-hot expansion; the
        # [slot, node, chunk] layout keeps the nid broadcast OFF the last
        # dim, which the DVE runs at 2x — measured 1.49us vs 2.81us/tile).
        # Emitted before load_library: Iota lives in the 'standard' Pool
        # library, dma_gather in 'mlp'.
        iota_t = consts.tile([P, P * CH_T], I16, tag="iota_t")
        nc.gpsimd.iota(
            iota_t[:].rearrange("p (n c) -> p n c", c=CH_T),
            pattern=[[1, P], [0, CH_T]],
            channel_multiplier=0,
        )
        nc.gpsimd.load_library(library_config.mlp)
        wcT_sb = consts.tile([XN_IN, XN_OUT], BF16, tag="wc")
        wnT_sb = consts.tile([XN_IN, XN_OUT], BF16, tag="wn")
        weT2_sb = consts.tile([P, XN_OUT], BF16, tag="we2")
        nc.sync.dma_start(wcT_sb[:], wcT[:, :])
        nc.sync.dma_start(wnT_sb[:], wnT[:, :])
        nc.sync.dma_start(weT2_sb[:], weT2[:, :])
        nid_all = consts.tile([P, n_tiles * CH_T], I16, tag="nid_all")
        nc.sync.dma_start(nid_all[:], nid_loc[:, :])
        # idx streams preloaded once, but as one DMA per group into disjoint
        # slices: the first gather gates only on its own group's slice, and
        # steady state carries no just-in-time idx dependency at all.
        idx_all = consts.tile([P, n_groups * NCLS * seg_i16], I16, tag="idx_all")
        for gg in range(n_groups):
            sl = slice(gg * NCLS * seg_i16, (gg + 1) * NCLS * seg_i16)
            nc.sync.dma_start(idx_all[:, sl], idx_loc[:, sl])
        # x viewed as [n_src/4, 4, 128] bf16: class j gathers row 4*i16+j via
        # elem_step=512 elements (1024B stride) and a j*128-element offset
        x4v = x4.rearrange("r (c f) -> r c f", c=NCLS)

        # Warm the 8 SWDGE bookkeeping lanes: each dummy absorbs the
        # idx-preload front so later gathers carry only their PE front.
        scratch = ctx.enter_context(tc.tile_pool(name="scratch", bufs=1))
        for q in range(8):
            sc = scratch.tile([1, K], I16, tag=f"sc{q}")
            nc.gpsimd.dma_start(sc[:], iota_t[:1, :K])
        # Tiny template instructions for _legalize_waits nop carriers
        # (one per DMA queue and per compute engine).
        nop_hw = scratch.tile([1, K], I16, tag="noptpl_hw")
        nc.sync.dma_start(nop_hw[:], idx_loc[:1, :K])
        nop_hwa = scratch.tile([1, K], I16, tag="noptpl_hwa")
        nc.scalar.dma_start(nop_hwa[:], idx_loc[:1, :K])
        nop_sw = scratch.tile([1, K], I16, tag="noptpl_sw")
        nc.gpsimd.dma_start(nop_sw[:], idx_loc[:1, :K])
        nop_dve = scratch.tile([P, K], I16, tag="noptpl_dve")
        nc.vector.tensor_copy(nop_dve[:], iota_t[:, :K])
        nop_act = scratch.tile([P, K], I16, tag="noptpl_act")
        nc.scalar.copy(nop_act[:], iota_t[:, :K])
        nop_pool = scratch.tile([P, K], F32, tag="noptpl_pool")
        nc.gpsimd.memset(nop_pool[:], 0.0)

        g_pool = ctx.enter_context(tc.tile_pool(name="gatherp", bufs=3))
        pp_pool = ctx.enter_context(tc.tile_pool(name="poolmat", bufs=2))
        e_pool = ctx.enter_context(tc.tile_pool(name="edgep", bufs=3))
        xs_pool = ctx.enter_context(tc.tile_pool(name="xselfp", bufs=2))
        st_pool = ctx.enter_context(tc.tile_pool(name="stagep", bufs=3))
        out_pool = ctx.enter_context(tc.tile_pool(name="outp", bufs=2))
        psum_pool = ctx.enter_context(tc.tile_pool(name="psump", bufs=2, space="PSUM"))
        psum1_pool = ctx.enter_context(tc.tile_pool(name="psum1p", bufs=2, space="PSUM"))

        # Warm up PE's view of the constant weights so steady-state matmuls
        # carry at most one sync wait (PE LDWEIGHTS supports a single wait).
        ps_warm = psum1_pool.tile([P, P], F32, tag="warm")
        nc.tensor.matmul(ps_warm[:], wcT_sb[:], wcT_sb[:], start=True, stop=False)
        nc.tensor.matmul(ps_warm[:], wnT_sb[:], wnT_sb[:], start=False, stop=False)
        nc.tensor.matmul(ps_warm[:], weT2_sb[:], weT2_sb[:], start=False, stop=True)

        t = 0
        gbf = [None] * NCLS
        nidx_reg = nc.gpsimd.to_reg(grp * SEG)  # shared across all gathers
        for c, ct in enumerate(chunks):
            o_stage = out_pool.tile([P, ct * XN_OUT], BF16, tag="ostage")
            for i in range(ct):
                g, ti = divmod(t, grp)

                if ti == 0:
                    # per-group gathers: one dma_gather per mod-4 class of
                    # grp*SEG slots; slot s lands at partition s%128, free
                    # block s//128, so 128-slot chunks stay within one tile.
                    for j in range(NCLS):
                        off = (g * NCLS + j) * seg_i16
                        gb = g_pool.tile(
                            [P, grp * SEG // P, XN_IN], BF16, tag=f"gb{j}"
                        )
                        nc.gpsimd.dma_gather(
                            out_ap=gb[:],
                            in_ap=x4v[:, j, :],
                            idxs_ap=idx_all[:, off:off + seg_i16],
                            num_idxs=grp * SEG,
                            num_idxs_reg=nidx_reg,
                            elem_size=XN_IN,
                            elem_step=NCLS * XN_IN,
                            single_packet=False,
                        )
                        gbf[j] = gb[:].rearrange("p b f -> p (b f)")

                # one-hot P[slot, n*CH_T+c] = (nid[slot, c] == n), bf16;
                # node-major layout so the nid broadcast is on the middle dim
                p_sb = pp_pool.tile([P, P * CH_T], BF16, tag="pmat")
                nid_b = nid_all[:, t * CH_T:(t + 1) * CH_T]
                in0, in1 = bass.broadcast_tensor_aps(
                    iota_t[:].rearrange("p (n c) -> p n c", c=CH_T),
                    nid_b.rearrange("p (one c) -> p one c", one=1),
                )
                nc.vector.tensor_tensor(
                    p_sb[:].rearrange("p (n c) -> p n c", c=CH_T),
                    in0, in1, op=mybir.AluOpType.is_equal,
                )
                p_view = p_sb[:].rearrange("p (n c) -> p c n", c=CH_T)

                # e arrives pre-permuted: e_sb[p=(par,f), kk*128+n]; the mean
                # over k happens on PE as 8 extra accumulating matmuls below.
                # Two tiles per DMA, issued on the ACT HWDGE queue to keep
                # the SP sequencer off the critical path.
                if ti == 0:
                    xT_sb = xs_pool.tile([P, grp * P], BF16, tag="xT")
                    nc.sync.dma_start(
                        xT_sb[:], xT_loc[:, t * P:(t + grp) * P]
                    )
                if t % 2 == 0:
                    e_sb = e_pool.tile([P, 2 * (K // 2) * P], BF16, tag="e")
                    nc.sync.dma_start(
                        e_sb[:],
                        eT_loc[:, t * (K // 2) * P:(t + 2) * (K // 2) * P],
                    )
                e_off = (t % 2) * (K // 2) * P

                # xnjT[f, n] = sum_slot g[slot, f] * P[slot, n]
                xnjT_ps = psum_pool.tile([P, P], F32, tag="ps_xnj")
                for b in range(CH_T):
                    j, bl = divmod(b, NCH)
                    blk = ti * NCH + bl
                    nc.tensor.matmul(
                        xnjT_ps[:],
                        gbf[j][:, blk * XN_IN:(blk + 1) * XN_IN],
                        p_view[:, b, :],
                        start=(b == 0),
                        stop=(b == CH_T - 1),
                    )
                xnjT_sb = st_pool.tile([P, P], BF16, tag="sb_xnj")
                nc.scalar.copy(xnjT_sb[:], xnjT_ps[:])

                out_ps = psum1_pool.tile([P, XN_OUT], F32, tag="ps_out")
                nc.tensor.matmul(
                    out_ps[:], xT_sb[:, ti * P:(ti + 1) * P], wcT_sb[:],
                    start=True, stop=False,
                )
                nc.tensor.matmul(out_ps[:], xnjT_sb[:], wnT_sb[:], start=False, stop=False)
                # the e-mean: 8 accumulating matmuls against weT2 (the
                # parity-stacked We.T/K); PSUM f32 accumulation
                for kk in range(K // 2):
                    nc.tensor.matmul(
                        out_ps[:],
                        e_sb[:, e_off + kk * P:e_off + (kk + 1) * P],
                        weT2_sb[:],
                        start=False, stop=(kk == K // 2 - 1),
                    )

                # ReLU (+cast to bf16) into the chunk staging buffer, on ACT
                nc.scalar.activation(
                    o_stage[:, i * XN_OUT:(i + 1) * XN_OUT], out_ps[:],
                    mybir.ActivationFunctionType.Relu,
                )
                t += 1

            nc.sync.dma_start(outs[c][:, :], o_stage[:])

    from concourse.library_overlay import lower_extended_insts

    lower_extended_insts(nc)
    _legalize_waits(nc, nop_sem)
    return nc


def _legalize_waits(nc: bass.Bass, nop_sem) -> None:
    """Split multi-wait queue-DMAs / matmuls for walrus's 1-wait codegen limit.

    The TRN2 walrus codegen allows a single sync-wait command per queue-DMA
    entry and per PE matmul (S3_LW struct). Tile emits minimal waits but can
    still produce 2+ (e.g. a slot's previous-writer DMA completion plus its
    last-reader engine release - Tile's clocks are not transitive). Queue
    entries execute in FIFO order, so extra waits are moved onto tiny no-op
    carrier DMAs inserted immediately before the offender on the same queue.
    For matmuls the carrier is a 1-column bf16 LDWEIGHTS (any clobbered
    weights are reloaded by each matmul's own weight load; insertion happens
    before a directly-preceding LDWEIGHTS so split LDW+MM pairs stay intact).
    """
    import copy

    dma_tpl: dict = {}
    eng_tpl: dict = {}
    evsem_tpl: dict = {}
    ldw_tpl = None
    for f in nc.m.functions:
        for blk in f.blocks:
            for inst in blk.instructions:
                tn = type(inst).__name__
                dst = (
                    str(getattr(inst.outs[0], "memref", "")) if inst.outs else ""
                )
                if tn == "InstDMACopy":
                    if dst.startswith("nop_hw") or dst.startswith("nop_sw"):
                        dma_tpl[inst.queue] = inst
                elif tn == "InstLdweights" and ldw_tpl is None:
                    ldw_tpl = inst
                elif tn == "InstEventSemaphore":
                    evsem_tpl[inst.engine] = inst
                elif dst.startswith("nop_dve") or dst.startswith("nop_act") or dst.startswith("nop_pool"):
                    eng_tpl[inst.engine] = inst

    counter = [0]

    def make_nop(tpl, wait):
        counter[0] += 1
        nop = copy.deepcopy(tpl)
        nop.name = f"I-{nc.next_id()}"
        # DMA carriers must update a semaphore (BIR invariant); use a
        # dedicated one nobody waits on. Other engines' carriers stay
        # update-free (walrus rejects a waitnop update on e.g. TensorCopy
        # with a no_semaphore_value_conflict ISA check).
        upd = []
        if type(tpl).__name__ == "InstDMACopy":
            upd = [
                mybir.SyncUpdate(
                    sync_type="semaphore",
                    id=nop_sem.num,
                    ant_name=nop_sem.name,
                    update_mode="sem-add-imm",
                    update_value=16,
                )
            ]
        nop.sync_info = mybir.SyncInfo(on_wait=[wait], on_update=upd)
        nc.inst_map[nop.name] = nop
        return nop

    for f in nc.m.functions:
        for blk in f.blocks:
            out: list = []
            changed = False
            insts = list(blk.instructions)
            for pos, inst in enumerate(insts):
                tn = type(inst).__name__
                si = inst.sync_info
                waits = list(si.on_wait) if si else []
                nops = None
                if len(waits) > 1:
                    if tn == "InstDMACopy":
                        tpl = dma_tpl.get(inst.queue)
                        assert tpl is not None, f"no nop template for {inst.queue}"
                        nops = [make_nop(tpl, w) for w in waits[:-1]]
                    elif tn in ("InstMatmult", "InstLdweights"):
                        assert ldw_tpl is not None, "no ldweights template"
                        nops = [make_nop(ldw_tpl, w) for w in waits[:-1]]
                        # keep split LDW+MM pairs adjacent
                        if out and type(out[-1]).__name__ == "InstLdweights":
                            own_ldw = out.pop()
                            nops.append(own_ldw)
                    elif tn == "InstDrain":
                        # a drain is its own carrier: extra single-wait drains
                        # on the same engine are harmless
                        nops = [make_nop(inst, w) for w in waits[:-1]]
                    elif inst.engine in eng_tpl and tn not in (
                        "InstDrain",
                        "InstEventSemaphore",
                        "InstSemaphoreOp",
                    ):
                        nops = [make_nop(eng_tpl[inst.engine], w) for w in waits[:-1]]
                if nops:
                    out.extend(nops)
                    inst.sync_info = mybir.SyncInfo(
                        on_wait=waits[-1:], on_update=list(si.on_update)
                    )
                    changed = True
                out.append(inst)
            if changed:
                try:
                    blk.instructions[:] = out
                except TypeError:
                    blk.instructions.clear()
                    blk.instructions.extend(out)


_PROGRAM_CACHE: dict = {}


def _get_program(n_loc_pad: int, n_src: int) -> bass.Bass:
    key = (n_loc_pad, n_src)
    if key not in _PROGRAM_CACHE:
        _PROGRAM_CACHE[key] = build_program(n_loc_pad, n_src)
    return _PROGRAM_CACHE[key]


def prep_gather(nh_pad: np.ndarray, grp: int):
    """Bucket edges by nh%4 per tile, emit int16 super-row ids (wrapped
    [16, L/16] layout replicated to 128 partitions) and per-slot node ids.

    Returns (idx16 [128, n_groups*NCLS*seg_i16], nid [128, n_tiles*CH_T] i16).
    """
    n_pad = nh_pad.shape[0]
    n_tiles = n_pad // P
    n_groups = n_tiles // grp
    seg_i16 = grp * SEG // 16

    idx16 = np.zeros((n_groups * NCLS, grp * SEG), np.int16)
    nid = np.full((n_tiles, CH_T, P), 255, np.int16)  # [tile, chunk, slot%128]
    nodes_tpl = np.repeat(np.arange(P, dtype=np.int16), K)  # edge -> node
    for t in range(n_tiles):
        nh_t = nh_pad[t * P:(t + 1) * P]          # [128 nodes, K]
        vals = nh_t.reshape(-1)                    # edge -> neighbor id
        cls = vals % NCLS
        g, ti = divmod(t, grp)
        for j in range(NCLS):
            sel = np.nonzero(cls == j)[0]
            l = len(sel)
            assert l <= SEG, f"class overflow {l} > {SEG}"
            idx16[g * NCLS + j, ti * SEG:ti * SEG + l] = (
                vals[sel] // NCLS
            ).astype(np.int16)
            # local slot s of class j -> tile chunk j*NCH + s//128, row s%128
            nid[t, j * NCH:j * NCH + (l + P - 1) // P].reshape(-1)[:l] = (
                nodes_tpl[sel]
            )
    # wrap idx16: entry i -> [i%16, i//16]; replicate 16-row block to 128
    idx16 = idx16.reshape(n_groups * NCLS, grp * SEG // 16, 16).transpose(0, 2, 1)
    idx16 = np.tile(idx16, (1, 8, 1)).reshape(n_groups, NCLS, P, seg_i16)
    idx16 = np.ascontiguousarray(
        idx16.transpose(2, 0, 1, 3).reshape(P, n_groups * NCLS * seg_i16)
    )
    # nid: [tile, chunk, slot] -> [slot(part), tile*CH_T + chunk]
    nid = np.ascontiguousarray(
        nid.transpose(2, 0, 1).reshape(P, n_tiles * CH_T)
    )
    return idx16, nid


def assemble_out(res_core: dict, n_tiles: int) -> np.ndarray:
    """Per-chunk partition-major bf16 outputs -> [n_loc_pad, 128] f32."""
    parts = []
    for c, ct in enumerate(_chunks(n_tiles)):
        o = np.asarray(res_core[f"out{c}"]).astype(np.float32)  # [128, ct*128]
        parts.append(
            o.reshape(P, ct, XN_OUT).transpose(1, 0, 2).reshape(ct * P, XN_OUT)
        )
    return np.concatenate(parts, axis=0)


def make_in_maps(x, e, ij, Wc, Wn, We, n_cores=N_CORES):
    """Host-side shard/prep: per-core input dicts for the SPMD program."""
    import ml_dtypes

    bf16 = ml_dtypes.bfloat16
    n = x.shape[0]
    n_loc = n // n_cores
    n_loc_pad = ((n_loc + P - 1) // P) * P
    n_tiles = n_loc_pad // P
    grp = GRP if n_tiles % GRP == 0 else (2 if n_tiles % 2 == 0 else 1)

    x = np.asarray(x, np.float32)
    assert n % NCLS == 0
    x4 = np.ascontiguousarray(x.reshape(n // NCLS, NCLS * XN_IN)).astype(bf16)
    nh = np.ascontiguousarray(ij[:, :, 0]).astype(np.int32)
    wcT = np.ascontiguousarray(Wc.T).astype(bf16)
    wnT = (np.ascontiguousarray(Wn.T) / np.float32(K)).astype(bf16)
    # weT2[(par*64+f), o] = We[o, f] / K   (both parity halves identical)
    weT2 = np.tile(np.ascontiguousarray(We.T) / np.float32(K), (2, 1)).astype(bf16)

    in_maps = []
    for c in range(n_cores):
        sl = slice(c * n_loc, (c + 1) * n_loc)
        xs = np.zeros((n_loc_pad, XN_IN), np.float32)
        xs[:n_loc] = x[sl]
        xT = np.ascontiguousarray(xs.T).astype(bf16)  # [128 f, n_loc_pad]
        # eT[(par*64+f), t, kk, n] = e[t*128+n, 2kk+par, f]
        e_c = np.zeros((n_loc_pad, K, XE_IN), np.float32)
        e_c[:n_loc] = np.asarray(e[sl], np.float32)
        eT = e_c.reshape(n_tiles, P, K // 2, 2, XE_IN)  # [t, n, kk, par, f]
        eT = np.ascontiguousarray(
            eT.transpose(3, 4, 0, 2, 1)  # [par, f, t, kk, n]
        ).reshape(P, n_tiles * (K // 2) * P).astype(bf16)
        # pad rows cycle 0..3 so no per-tile mod-class bucket overflows SEG
        idx_c = np.tile(np.arange(K, dtype=np.int32) % NCLS, (n_loc_pad, 1))
        idx_c[:n_loc] = nh[sl]
        idx16, nid = prep_gather(idx_c, grp)
        in_maps.append(
            {
                "x4": x4,
                "xT_loc": xT,
                "eT_loc": eT,
                "idx_loc": idx16,
                "nid_loc": nid,
                "wcT": wcT,
                "wnT": wnT,
                "weT2": weT2,
            }
        )
    return in_maps, n_loc, n_loc_pad


def kernel(x, e, ij, Wc, Wn, We):
    x = np.asarray(x)
    e = np.asarray(e)
    ij = np.asarray(ij)
    in_maps, n_loc, n_loc_pad = make_in_maps(x, e, ij, Wc, Wn, We)
    nc = _get_program(n_loc_pad, x.shape[0])
    res = run_bass_kernel_spmd(nc, in_maps, list(range(N_CORES)))
    n_tiles = n_loc_pad // P
    out = np.concatenate(
        [assemble_out(r, n_tiles)[:n_loc] for r in res.results], axis=0
    )
    return out.astype(np.float32)
